# revision 19
# baseline (speedup 1.0000x reference)
"""Trainium2 Bass kernel for nn_ClusteringLayer (vq_codebook, Student-t assignments).

Computes, for x in R^{N x D} and clusters c in R^{K x D}:
    d2[n,k] = ||x_n - c_k||^2
    q = 1 / (1 + d2)            (Student-t, alpha=1, power=(alpha+1)/2=1)
    out = q / q.sum(-1, keepdims=True)

Strategy (data-parallel over 8 NeuronCores, cluster table replicated):
  - host: shard x along N (65536 rows/core); quantize x to fp8e4 (TRN e4m3,
    clip +-240) in DoubleRow layout [128, nslab, 2, slab] (d-halves as the
    two DR k-tiles); precompute 2-level-fp8 x2 residuals and the exact-f32
    per-k bias 1 + ||c_k||^2.
  - device, per 2048-column block, psum packed [2 groups x 64 k, 1024]:
      8 fp8 DoubleRow main matmuls (contraction 256 in one pass, 256 out
      cols each) + 8 fp8 DoubleRow aug matmuls (x2[n] via 2-level residual
      on the two k-tiles; all-zero other rows).
      u  = Ln(psum + bias_c2p1)   (ACT, f32 u; bias adds 1+c2[k] exactly)
      q  = Exp(-u)  -> f16
      s  = sel-matmul over k-partitions -> [2x2, 512] psums (f16 path)
      sinv = reciprocal_approx_fast (DVE)
      bcast = selT-matmuls; out = q * bcast (DVE) accumulated in slab tile
  - slab-grained (8192 cols) input/output DMAs for large descriptors.
  - host: upcast f16 -> f32, unpack [k, n] blocks to [n, k], concat shards.
"""

import numpy as np
from contextlib import ExitStack


def _patch_act_tables():
    """Make Ln and Exp resolve to the single set that contains both
    (natural_log_exp_and_others), so the kernel pays one ACT_TABLE_LOAD
    instead of alternating sets every block.  Only values are modified --
    set order (and hence act_func_set_id indices) is preserved."""
    import functools
    from concourse import hw_specs, bacc, mybir

    if getattr(hw_specs, "_act_tables_patched", False):
        return
    orig = hw_specs.get_activation_tables

    @functools.cache
    def patched(arch):
        t = dict(orig(arch))
        ln = mybir.ActivationFunctionType.Ln
        ex = mybir.ActivationFunctionType.Exp
        out = {}
        for name, funcs in t.items():
            if name != "natural_log_exp_and_others" and (ln in funcs or ex in funcs):
                funcs = funcs - {ln, ex}
            out[name] = funcs
        return out

    hw_specs.get_activation_tables = patched
    bacc.get_activation_tables = patched
    hw_specs._act_tables_patched = True

N, D, K = 524288, 256, 64
NCORES = 8
NSH = N // NCORES      # 65536 rows per core
BLK = 2048             # n-columns per psum block (2 groups x 1024)
SLAB = 8192            # n-columns per DMA slab (4 blocks)
NBLK = NSH // BLK      # 32
NSLAB = NSH // SLAB    # 8
BPS = SLAB // BLK      # blocks per slab = 4


def _build(nsh=NSH, blk=BLK, slab=SLAB):
    import concourse.bacc as bacc
    import concourse.tile as tile
    from concourse import mybir

    _patch_act_tables()

    f32 = mybir.dt.float32
    f16 = mybir.dt.float16
    f8 = mybir.dt.float8e4
    DR = mybir.MatmulPerfMode.DoubleRow
    nblk = nsh // blk
    bps = slab // blk
    nslab = nsh // slab
    half = blk // 2            # 1024 = columns per psum group
    qcols = half               # q/psum free size
    outw = bps * half          # out-slab width (1024 per block)

    nc = bacc.Bacc("TRN2", target_bir_lowering=False, debug=False)
    xt = nc.dram_tensor("xt", [128, nsh * 2], f8, kind="ExternalInput").ap()
    augm = nc.dram_tensor("augm", [1, nsh * 2], f8, kind="ExternalInput").ap()
    # DoubleRow demands full-128-partition dst (ISA col_grp==0xf): each
    # group gets a full-width stationary with the other 64 out-columns
    # zeroed; both groups accumulate into the same [128, 256] psum region.
    ctb = nc.dram_tensor("ctb", [128, 512], f8, kind="ExternalInput").ap()
    augw = nc.dram_tensor("augw", [128, 512], f8, kind="ExternalInput").ap()
    biasv = nc.dram_tensor("biasv", [128, 1], f32, kind="ExternalInput").ap()
    sel = nc.dram_tensor("sel", [128, 32], f16, kind="ExternalInput").ap()
    selt = nc.dram_tensor("selt", [128, 128], f16, kind="ExternalInput").ap()
    selt2 = nc.dram_tensor("selt2", [128, 128], f16, kind="ExternalInput").ap()
    qo = nc.dram_tensor("q", [128, nsh // 2], f16, kind="ExternalOutput").ap()

    xt_r = xt.rearrange("p (s i n) -> p s i n", s=nslab, i=2)
    augm_r = augm.rearrange("p (s i n) -> p s i n", s=nslab, i=2)

    with tile.TileContext(nc) as tc, ExitStack() as ctx, \
            nc.allow_low_precision(reason="fp8 gemm validated against reference"):
        wp = ctx.enter_context(tc.tile_pool(name="w", bufs=1))
        io = ctx.enter_context(tc.tile_pool(name="io", bufs=5))
        up = ctx.enter_context(tc.tile_pool(name="up", bufs=3))
        qp = ctx.enter_context(tc.tile_pool(name="qp", bufs=7))
        sv = ctx.enter_context(tc.tile_pool(name="sv", bufs=3))
        op = ctx.enter_context(tc.tile_pool(name="op", bufs=3))
        pp = ctx.enter_context(tc.tile_pool(name="ps", bufs=2, space="PSUM"))
        sp = ctx.enter_context(tc.tile_pool(name="sp", bufs=2, space="PSUM"))
        bp = ctx.enter_context(tc.tile_pool(name="bp", bufs=1, space="PSUM"))

        # one-time constants
        ctb_sb = wp.tile([128, 2, 2, 128], f8, tag="ctb")   # [d, A/B, i, m]
        nc.sync.dma_start(ctb_sb, ctb.rearrange("p (a i m) -> p a i m", a=2, i=2))
        augw_sb = wp.tile([128, 2, 2, 128], f8, tag="augw")
        nc.sync.dma_start(augw_sb, augw.rearrange("p (a i m) -> p a i m", a=2, i=2))
        biasv_sb = wp.tile([128, 1], f32, tag="biasv")
        nc.sync.dma_start(biasv_sb, biasv)
        sel_sb = wp.tile([128, 32], f16, tag="sel")
        nc.sync.dma_start(sel_sb, sel)
        selt_sb = wp.tile([128, 128], f16, tag="selt")
        nc.sync.dma_start(selt_sb, selt)
        selt2_sb = wp.tile([128, 128], f16, tag="selt2")
        nc.sync.dma_start(selt2_sb, selt2)

        # Manually double-buffered aug moving tiles whose rows 1:128 are
        # multiplied by stationary zeros: memset once so the garbage rows
        # can never be NaN (0 * NaN = NaN would poison the psum).
        # fast-path tile for blocks 0-1: its short memset (+tiny DMA)
        # unblocks block 0's aug-matmul before the big slab-sized
        # aug buffers are ready.
        amfirst = wp.tile([128, 2, 2 * blk], f8, tag="amf")
        nc.vector.memset(amfirst, 0.0)
        ams = [wp.tile([128, 2, slab], f8, tag=f"am{j}", name=f"am{j}")
               for j in range(2)]
        nc.vector.memset(ams[0], 0.0)
        nc.gpsimd.memset(ams[1], 0.0)
        sis = [wp.tile([128, 512], f16, tag=f"si{j}", name=f"si{j}")
               for j in range(4)]
        for j in range(4):
            nc.vector.memset(sis[j], 0.0)

        xs = [None] * nslab    # x8 slab tile per live slab
        ams_cur = [None] * nslab
        qs = [None] * nblk     # q tile per live block
        svs = [None] * nblk    # sinv tile per live block
        outs = [None] * nslab  # out slab tile

        def load_slab(s, split=False):
            x8 = io.tile([128, 2, slab], f8, tag="x8")
            if split:
                for bi in range(bps):
                    cs = slice(bi * blk, (bi + 1) * blk)
                    nc.sync.dma_start(x8[:, :, cs], xt_r[:, s, :, cs])
            else:
                nc.sync.dma_start(x8, xt_r[:, s])
            xs[s] = x8

        def load_aug(s):
            # ams are manually double-buffered: this DMA must be emitted
            # only after every aug-MM read of slab s-2 (same buffer) is
            # already in the program, or the scheduler will order those
            # reads AFTER this write.
            am = ams[s % 2]
            nc.sync.dma_start(am[0:1, :, :], augm_r[0:1, s])
            ams_cur[s] = am

        def main_mms(b):
            s, bi = b // bps, b % bps
            x8 = xs[s]
            am = amfirst if b < 2 else ams_cur[s]
            ps = pp.tile([128, half], f32, tag="d2")
            # 8 full-width 512-col DR matmuls per block (moving free 1024),
            # grouped by stationary: 2 weight loads for mains, 2 for augs.
            # start=True pends the WHOLE 2KB psum bank to zero, so only the
            # first MM touching each bank may carry it (each MM = one bank).
            for a in range(2):          # a=0: group-0 cols, a=1: group-1
                for t in range(2):
                    csl = slice(t * 512, (t + 1) * 512)
                    xoff = bi * blk + a * half + t * 512
                    xsl = slice(xoff, xoff + 512)
                    nc.tensor.matmul(ps[:, csl], ctb_sb[:, a], x8[:, :, xsl],
                                     start=(a == 0), stop=False, perf_mode=DR)
            for a in range(2):
                for t in range(2):
                    csl = slice(t * 512, (t + 1) * 512)
                    xoff = bi * blk + a * half + t * 512
                    xsl = slice(xoff, xoff + 512)
                    nc.tensor.matmul(ps[:, csl], augw_sb[:, a], am[:, :, xsl],
                                     start=False, stop=(a == 1), perf_mode=DR)
            # q = 1/(1+d2) = exp(-ln(psum + (1+c2[k]))) on the ACT engine;
            # u kept f32 to avoid the f16 ulp(u) hit on q.
            u = up.tile([128, qcols], f32, tag="u")
            nc.scalar.activation(u, ps, func=mybir.ActivationFunctionType.Ln,
                                 bias=biasv_sb, scale=1.0)
            q = qp.tile([128, qcols], f16, tag="q")
            nc.scalar.activation(q, u, func=mybir.ActivationFunctionType.Exp,
                                 scale=-1.0)
            qs[b] = q

        def norm_a(b):
            # both 512-halves' group sums packed into one [34, 512] psum:
            # h0 at rows 0:2, h1 at rows 32:34 (base-partition-32 aligned),
            # so one approx + one cast cover both.
            # sel's columns 2:32 are all-ones dummies so rows 2:32 of the
            # psum hold finite sums -- the packed reciprocal below must not
            # see stale/zero psum (1/0 -> inf -> 0*inf = NaN at the bcast).
            q = qs[b]
            st = sp.tile([64, 512], f32, tag="sh")
            nc.tensor.matmul(st[0:32, :], sel_sb, q[:, 0:512],
                             start=True, stop=True)
            nc.tensor.matmul(st[32:64, :], sel_sb, q[:, 512:1024],
                             start=True, stop=True, tile_position=(0, 32))
            sf = sv.tile([34, 512], f32, tag="sf")
            nc.vector.reciprocal_approx_fast(sf, st[0:34, :])
            si = sis[b % 4]
            nc.vector.tensor_copy(si[0:34, :], sf)
            svs[b] = si

        def norm_b(b):
            s, bi = b // bps, b % bps
            si = svs[b]
            bc = bp.tile([128, half], f32, tag="bc")
            nc.tensor.matmul(bc[:, 0:512], selt_sb, si, start=True, stop=True)
            nc.tensor.matmul(bc[:, 512:1024], selt2_sb, si,
                             start=True, stop=True)
            if bi == 0:
                ot = op.tile([128, outw], f16, tag="out")
                outs[s] = ot
            nc.vector.tensor_tensor(outs[s][:, bi * half:(bi + 1) * half],
                                    qs[b], bc, op=mybir.AluOpType.mult)
            if s == nslab - 1:
                # last slab: stream the output per block so the final DMA
                # overlaps the pipeline drain instead of following it.
                o0 = s * outw + bi * half
                nc.scalar.dma_start(qo[:, o0:o0 + half],
                                    outs[s][:, bi * half:(bi + 1) * half])
            elif bi == bps - 1:
                nc.scalar.dma_start(
                    qo[:, s * outw:(s + 1) * outw], outs[s])

        nc.sync.dma_start(amfirst[0:1, :, :], augm_r[0:1, 0, :, 0:2 * blk])
        load_slab(0, split=True)
        load_aug(0)
        if nslab > 1:
            load_slab(1)
            load_aug(1)
        if nslab > 2:
            load_slab(2)

        # mains(i) are emitted FIRST so the in-order tensor queue never
        # parks a ready main matmul behind a norm stage that still waits
        # on the scalar/vector chain; norm stages run 3 and 5 blocks
        # behind to decouple the PE from the ACT->DVE latency.
        for i in range(nblk + 5):
            if i < nblk and i % bps == 0 and i // bps + 3 < nslab:
                load_slab(i // bps + 3)
            if i < nblk:
                main_mms(i)
                if (i + 1) % bps == 0 and i // bps + 2 < nslab:
                    load_aug(i // bps + 2)
            if 5 <= i:
                norm_b(i - 5)
            if 3 <= i and i - 3 < nblk:
                norm_a(i - 3)

    nc.compile()
    return nc


_CACHE = {}


def _get_nc():
    if "nc" not in _CACHE:
        _CACHE["nc"] = _build()
    return _CACHE["nc"]


def _prep_inputs(x, c):
    """Build per-core input maps (host-side shard + layout prep)."""
    import ml_dtypes

    f16 = np.float16
    f8 = ml_dtypes.float8_e4m3
    x = np.asarray(x, dtype=np.float32)
    c = np.asarray(c, dtype=np.float32)
    assert x.shape == (N, D) and c.shape == (K, D)

    def q8(a):
        return np.clip(a, -240.0, 240.0).astype(f8)

    c2p1 = 1.0 + np.sum(c * c, axis=1)                     # (K,)
    biasv = np.empty((128, 1), np.float32)
    biasv[0:K, 0] = c2p1
    biasv[K:128, 0] = c2p1
    # ctb[d, a, i, m]: group-a stationary, DR k-tiles i = d-halves,
    # out-columns m: group a's 64 k's live at m in [a*64, a*64+64).
    ct = (-2.0 * c).T.reshape(2, 128, K).transpose(1, 0, 2)  # [d, i, k]
    ctb = np.zeros((128, 2, 2, 128), np.float32)
    ctb[:, 0, :, 0:K] = ct
    ctb[:, 1, :, K:128] = ct
    ctb = q8(ctb.reshape(128, 512))
    # aug weights: row d=0 only: ktile0 scale 2, ktile1 scale 1
    augw = np.zeros((128, 2, 2, 128), np.float32)
    augw[0, 0, 0, 0:K] = 2.0
    augw[0, 0, 1, 0:K] = 1.0
    augw[0, 1, 0, K:128] = 2.0
    augw[0, 1, 1, K:128] = 1.0
    augw = q8(augw.reshape(128, 512))
    sel = np.ones((128, 32), f16)      # cols 2:32 = dummy all-ones columns
    sel[:, 0] = 0.0
    sel[:, 1] = 0.0
    sel[0:K, 0] = 1.0
    sel[K:128, 1] = 1.0
    selt = np.zeros((128, 128), f16)                       # K=128 zero-padded
    selt[0, 0:K] = 1.0
    selt[1, K:128] = 1.0
    selt2 = np.zeros((128, 128), f16)                      # h1 variant (rows 32/33)
    selt2[32, 0:K] = 1.0
    selt2[33, K:128] = 1.0

    x8full = q8(x)                                         # (N, 256) fp8
    x2 = np.sum(x.astype(np.float32) ** 2, axis=1)         # (N,) f32
    r0 = q8(x2 * 0.5)                                      # 2-level fp8 x2:
    r1 = q8(x2 - 2.0 * r0.astype(np.float32))              # 2*r0 + r1 ~ x2

    in_maps = []
    for i in range(NCORES):
        sl = slice(i * NSH, (i + 1) * NSH)
        # [d, s, i, n] layout: DR k-tiles (d-halves) at stride SLAB
        xts = np.ascontiguousarray(
            x8full[sl].T.reshape(2, 128, NSLAB, SLAB)
            .transpose(1, 2, 0, 3).reshape(128, 2 * NSH))
        am = np.empty((1, NSLAB, 2, SLAB), f8)
        am[0, :, 0, :] = r0[sl].reshape(NSLAB, SLAB)
        am[0, :, 1, :] = r1[sl].reshape(NSLAB, SLAB)
        in_maps.append({"xt": xts, "augm": am.reshape(1, 2 * NSH),
                        "ctb": ctb, "augw": augw, "biasv": biasv,
                        "sel": sel, "selt": selt, "selt2": selt2})
    return in_maps


def _postprocess(results):
    """[128, NSH/2] f16 per core -> full [N, K] f32."""
    outs = []
    for r in results:
        qt = np.asarray(r["q"]).astype(np.float32)          # [128, 32768]
        a = qt.reshape(2, K, NSLAB, BPS, BLK // 2)          # [g, k, s, bi, j]
        outs.append(a.transpose(2, 3, 0, 4, 1).reshape(NSH, K))
    return np.concatenate(outs, axis=0)


def kernel(inputs, clusters):
    from concourse.bass_utils import run_bass_kernel_spmd

    nc = _get_nc()
    in_maps = _prep_inputs(inputs, clusters)
    res = run_bass_kernel_spmd(nc, in_maps, core_ids=list(range(NCORES)))
    return _postprocess(res.results)


# revision 25
# speedup vs baseline: 1.0379x; 1.0379x over previous
"""Trainium2 Bass kernel for nn_ClusteringLayer (vq_codebook, Student-t assignments).

Computes, for x in R^{N x D} and clusters c in R^{K x D}:
    d2[n,k] = ||x_n - c_k||^2
    q = 1 / (1 + d2)            (Student-t, alpha=1, power=(alpha+1)/2=1)
    out = q / q.sum(-1, keepdims=True)

Strategy (data-parallel over 8 NeuronCores, cluster table replicated):
  - host: shard x along N (65536 rows/core); quantize x to fp8e4 (TRN e4m3,
    clip +-240) in DoubleRow layout [128, nslab, 2, slab] (d-halves as the
    two DR k-tiles); precompute 2-level-fp8 x2 residuals and the exact-f32
    per-k bias 1 + ||c_k||^2.
  - device, per 2048-column block, psum packed [2 groups x 64 k, 1024]:
      8 fp8 DoubleRow main matmuls (contraction 256 in one pass, 256 out
      cols each) + 8 fp8 DoubleRow aug matmuls (x2[n] via 2-level residual
      on the two k-tiles; all-zero other rows).
      u  = Ln(psum + bias_c2p1)   (ACT, f32 u; bias adds 1+c2[k] exactly)
      q  = Exp(-u)  -> f16
      s  = sel-matmul over k-partitions -> [2x2, 512] psums (f16 path)
      sinv = reciprocal_approx_fast (DVE)
      bcast = selT-matmuls; out = q * bcast (DVE) accumulated in slab tile
  - slab-grained (8192 cols) input/output DMAs for large descriptors.
  - host: upcast f16 -> f32, unpack [k, n] blocks to [n, k], concat shards.
"""

import numpy as np
from contextlib import ExitStack


def _patch_act_tables():
    """Make Ln and Exp resolve to the single set that contains both
    (natural_log_exp_and_others), so the kernel pays one ACT_TABLE_LOAD
    instead of alternating sets every block.  Only values are modified --
    set order (and hence act_func_set_id indices) is preserved."""
    import functools
    from concourse import hw_specs, bacc, mybir

    if getattr(hw_specs, "_act_tables_patched", False):
        return
    orig = hw_specs.get_activation_tables

    @functools.cache
    def patched(arch):
        t = dict(orig(arch))
        ln = mybir.ActivationFunctionType.Ln
        ex = mybir.ActivationFunctionType.Exp
        out = {}
        for name, funcs in t.items():
            if name != "natural_log_exp_and_others" and (ln in funcs or ex in funcs):
                funcs = funcs - {ln, ex}
            out[name] = funcs
        return out

    hw_specs.get_activation_tables = patched
    bacc.get_activation_tables = patched
    hw_specs._act_tables_patched = True

N, D, K = 524288, 256, 64
NCORES = 8
NSH = N // NCORES      # 65536 rows per core
BLK = 2048             # n-columns per psum block (2 groups x 1024)
SLAB = 8192            # n-columns per DMA slab (4 blocks)
NBLK = NSH // BLK      # 32
NSLAB = NSH // SLAB    # 8
BPS = SLAB // BLK      # blocks per slab = 4


def _build(nsh=NSH, blk=BLK, slab=SLAB):
    import concourse.bacc as bacc
    import concourse.tile as tile
    from concourse import mybir

    _patch_act_tables()

    f32 = mybir.dt.float32
    f16 = mybir.dt.float16
    f8 = mybir.dt.float8e4
    DR = mybir.MatmulPerfMode.DoubleRow
    nblk = nsh // blk
    bps = slab // blk
    nslab = nsh // slab
    half = blk // 2            # 1024 = columns per psum group
    qcols = half               # q/psum free size
    outw = bps * half          # out-slab width (1024 per block)

    nc = bacc.Bacc("TRN2", target_bir_lowering=False, debug=False)
    xt = nc.dram_tensor("xt", [128, nsh * 2], f8, kind="ExternalInput").ap()
    augm = nc.dram_tensor("augm", [1, nsh * 2], f8, kind="ExternalInput").ap()
    # DoubleRow demands full-128-partition dst (ISA col_grp==0xf): each
    # group gets a full-width stationary with the other 64 out-columns
    # zeroed; both groups accumulate into the same [128, 256] psum region.
    ctb = nc.dram_tensor("ctb", [128, 512], f8, kind="ExternalInput").ap()
    augw = nc.dram_tensor("augw", [128, 512], f8, kind="ExternalInput").ap()
    biasv = nc.dram_tensor("biasv", [128, 1], f32, kind="ExternalInput").ap()
    sel = nc.dram_tensor("sel", [128, 32], f16, kind="ExternalInput").ap()
    selt = nc.dram_tensor("selt", [128, 128], f16, kind="ExternalInput").ap()
    selt2 = nc.dram_tensor("selt2", [128, 128], f16, kind="ExternalInput").ap()
    qo = nc.dram_tensor("q", [128, nsh // 2], f16, kind="ExternalOutput").ap()

    xt_r = xt.rearrange("p (s i n) -> p s i n", s=nslab, i=2)
    augm_r = augm.rearrange("p (s i n) -> p s i n", s=nslab, i=2)

    with tile.TileContext(nc) as tc, ExitStack() as ctx, \
            nc.allow_low_precision(reason="fp8 gemm validated against reference"):
        wp = ctx.enter_context(tc.tile_pool(name="w", bufs=1))
        io = ctx.enter_context(tc.tile_pool(name="io", bufs=5))
        up = ctx.enter_context(tc.tile_pool(name="up", bufs=3))
        qp = ctx.enter_context(tc.tile_pool(name="qp", bufs=9))
        sv = ctx.enter_context(tc.tile_pool(name="sv", bufs=3))
        op = ctx.enter_context(tc.tile_pool(name="op", bufs=3))
        pp = ctx.enter_context(tc.tile_pool(name="ps", bufs=2, space="PSUM"))
        sp = ctx.enter_context(tc.tile_pool(name="sp", bufs=1, space="PSUM"))
        bp = ctx.enter_context(tc.tile_pool(name="bp", bufs=3, space="PSUM"))

        # one-time constants
        ctb_sb = wp.tile([128, 2, 2, 128], f8, tag="ctb")   # [d, A/B, i, m]
        nc.sync.dma_start(ctb_sb, ctb.rearrange("p (a i m) -> p a i m", a=2, i=2))
        augw_sb = wp.tile([128, 2, 2, 128], f8, tag="augw")
        nc.sync.dma_start(augw_sb, augw.rearrange("p (a i m) -> p a i m", a=2, i=2))
        biasv_sb = wp.tile([128, 1], f32, tag="biasv")
        nc.sync.dma_start(biasv_sb, biasv)
        sel_sb = wp.tile([128, 32], f16, tag="sel")
        nc.sync.dma_start(sel_sb, sel)
        selt_sb = wp.tile([128, 128], f16, tag="selt")
        nc.sync.dma_start(selt_sb, selt)
        selt2_sb = wp.tile([128, 128], f16, tag="selt2")
        nc.sync.dma_start(selt2_sb, selt2)

        # Manually double-buffered aug moving tiles whose rows 1:128 are
        # multiplied by stationary zeros: memset once so the garbage rows
        # can never be NaN (0 * NaN = NaN would poison the psum).
        # fast-path tile for blocks 0-1: its short memset (+tiny DMA)
        # unblocks block 0's aug-matmul before the big slab-sized
        # aug buffers are ready.
        amfirst = wp.tile([128, 2, 2 * blk], f8, tag="amf")
        nc.vector.memset(amfirst, 0.0)
        ams = [wp.tile([128, 2, slab], f8, tag=f"am{j}", name=f"am{j}")
               for j in range(2)]
        nc.vector.memset(ams[0], 0.0)
        nc.gpsimd.memset(ams[1], 0.0)
        sis = [wp.tile([128, 512], f16, tag=f"si{j}", name=f"si{j}")
               for j in range(6)]
        for j in range(6):
            nc.vector.memset(sis[j], 0.0)

        xs = [None] * nslab    # x8 slab tile per live slab
        ams_cur = [None] * nslab
        qs = [None] * nblk     # q tile per live block
        svs = [None] * nblk    # sinv tile per live block
        outs = [None] * nslab  # out slab tile

        def load_slab(s, split=False):
            x8 = io.tile([128, 2, slab], f8, tag="x8")
            if split:
                for bi in range(bps):
                    cs = slice(bi * blk, (bi + 1) * blk)
                    nc.sync.dma_start(x8[:, :, cs], xt_r[:, s, :, cs])
            else:
                nc.sync.dma_start(x8, xt_r[:, s])
            xs[s] = x8

        def load_aug(s):
            # ams are manually double-buffered: this DMA must be emitted
            # only after every aug-MM read of slab s-2 (same buffer) is
            # already in the program, or the scheduler will order those
            # reads AFTER this write.
            am = ams[s % 2]
            nc.sync.dma_start(am[0:1, :, :], augm_r[0:1, s])
            ams_cur[s] = am

        def main_mms(b):
            s, bi = b // bps, b % bps
            x8 = xs[s]
            am = amfirst if b < 2 else ams_cur[s]
            ps = pp.tile([128, half], f32, tag="d2")
            # 8 full-width 512-col DR matmuls per block (moving free 1024),
            # grouped by stationary: 2 weight loads for mains, 2 for augs.
            # start=True pends the WHOLE 2KB psum bank to zero, so only the
            # first MM touching each bank may carry it (each MM = one bank).
            for a in range(2):          # a=0: group-0 cols, a=1: group-1
                for t in range(2):
                    csl = slice(t * 512, (t + 1) * 512)
                    xoff = bi * blk + a * half + t * 512
                    xsl = slice(xoff, xoff + 512)
                    nc.tensor.matmul(ps[:, csl], ctb_sb[:, a], x8[:, :, xsl],
                                     start=(a == 0), stop=False, perf_mode=DR)
            for a in range(2):
                for t in range(2):
                    csl = slice(t * 512, (t + 1) * 512)
                    xoff = bi * blk + a * half + t * 512
                    xsl = slice(xoff, xoff + 512)
                    nc.tensor.matmul(ps[:, csl], augw_sb[:, a], am[:, :, xsl],
                                     start=False, stop=(a == 1), perf_mode=DR)
            # q = 1/(1+d2) = exp(-ln(psum + (1+c2[k]))) on the ACT engine;
            # u kept f32 to avoid the f16 ulp(u) hit on q.
            u = up.tile([128, qcols], f32, tag="u")
            nc.scalar.activation(u, ps, func=mybir.ActivationFunctionType.Ln,
                                 bias=biasv_sb, scale=1.0)
            q = qp.tile([128, qcols], f16, tag="q")
            nc.scalar.activation(q, u, func=mybir.ActivationFunctionType.Exp,
                                 scale=-1.0)
            qs[b] = q

        def norm_a(b):
            # both 512-halves' group sums packed into one [34, 512] psum:
            # h0 at rows 0:2, h1 at rows 32:34 (base-partition-32 aligned),
            # so one approx + one cast cover both.
            # sel's columns 2:32 are all-ones dummies so rows 2:32 of the
            # psum hold finite sums -- the packed reciprocal below must not
            # see stale/zero psum (1/0 -> inf -> 0*inf = NaN at the bcast).
            q = qs[b]
            st = sp.tile([64, 512], f32, tag="sh")
            nc.tensor.matmul(st[0:32, :], sel_sb, q[:, 0:512],
                             start=True, stop=True)
            nc.tensor.matmul(st[32:64, :], sel_sb, q[:, 512:1024],
                             start=True, stop=True, tile_position=(0, 32))
            sf = sv.tile([34, 512], f32, tag="sf")
            nc.vector.reciprocal_approx_fast(sf, st[0:34, :])
            si = sis[b % 6]
            nc.vector.tensor_copy(si[0:34, :], sf)
            svs[b] = si

        def norm_b(b):
            s, bi = b // bps, b % bps
            si = svs[b]
            if bi == 0:
                ot = op.tile([128, outw], f16, tag="out")
                outs[s] = ot
            # bc split into per-bank halves with a 3-deep pool so this MM
            # never waits on the previous block's DVE multiply (the bc-
            # buffer reuse was a zero-slack tensor<-vector coupling).
            for h, selw in ((0, selt_sb), (1, selt2_sb)):
                bc = bp.tile([128, 512], f32, tag="bc")
                nc.tensor.matmul(bc, selw, si, start=True, stop=True)
                nc.vector.tensor_tensor(
                    outs[s][:, bi * half + h * 512:bi * half + (h + 1) * 512],
                    qs[b][:, h * 512:(h + 1) * 512], bc,
                    op=mybir.AluOpType.mult)
            if s == nslab - 1:
                # last slab: stream the output per block so the final DMA
                # overlaps the pipeline drain instead of following it.
                o0 = s * outw + bi * half
                nc.scalar.dma_start(qo[:, o0:o0 + half],
                                    outs[s][:, bi * half:(bi + 1) * half])
            elif bi == bps - 1:
                nc.scalar.dma_start(
                    qo[:, s * outw:(s + 1) * outw], outs[s])

        nc.sync.dma_start(amfirst[0:1, :, :], augm_r[0:1, 0, :, 0:2 * blk])
        load_slab(0, split=True)
        load_aug(0)
        if nslab > 1:
            load_slab(1)
            load_aug(1)
        if nslab > 2:
            load_slab(2)

        # mains(i) are emitted FIRST so the in-order tensor queue never
        # parks a ready main matmul behind a norm stage that still waits
        # on the scalar/vector chain; norm stages run 3 and 5 blocks
        # behind to decouple the PE from the ACT->DVE latency.
        for i in range(nblk + 7):
            if i < nblk and i % bps == 0 and i // bps + 3 < nslab:
                load_slab(i // bps + 3)
            if i < nblk:
                main_mms(i)
                if (i + 1) % bps == 0 and i // bps + 2 < nslab:
                    load_aug(i // bps + 2)
            if 7 <= i:
                norm_b(i - 7)
            if 3 <= i and i - 3 < nblk:
                norm_a(i - 3)

    nc.compile()
    return nc


_CACHE = {}


def _get_nc():
    if "nc" not in _CACHE:
        _CACHE["nc"] = _build()
    return _CACHE["nc"]


def _prep_inputs(x, c):
    """Build per-core input maps (host-side shard + layout prep)."""
    import ml_dtypes

    f16 = np.float16
    f8 = ml_dtypes.float8_e4m3
    x = np.asarray(x, dtype=np.float32)
    c = np.asarray(c, dtype=np.float32)
    assert x.shape == (N, D) and c.shape == (K, D)

    def q8(a):
        return np.clip(a, -240.0, 240.0).astype(f8)

    c2p1 = 1.0 + np.sum(c * c, axis=1)                     # (K,)
    biasv = np.empty((128, 1), np.float32)
    biasv[0:K, 0] = c2p1
    biasv[K:128, 0] = c2p1
    # ctb[d, a, i, m]: group-a stationary, DR k-tiles i = d-halves,
    # out-columns m: group a's 64 k's live at m in [a*64, a*64+64).
    ct = (-2.0 * c).T.reshape(2, 128, K).transpose(1, 0, 2)  # [d, i, k]
    ctb = np.zeros((128, 2, 2, 128), np.float32)
    ctb[:, 0, :, 0:K] = ct
    ctb[:, 1, :, K:128] = ct
    ctb = q8(ctb.reshape(128, 512))
    # aug weights: row d=0 only: ktile0 scale 2, ktile1 scale 1
    augw = np.zeros((128, 2, 2, 128), np.float32)
    augw[0, 0, 0, 0:K] = 2.0
    augw[0, 0, 1, 0:K] = 1.0
    augw[0, 1, 0, K:128] = 2.0
    augw[0, 1, 1, K:128] = 1.0
    augw = q8(augw.reshape(128, 512))
    sel = np.ones((128, 32), f16)      # cols 2:32 = dummy all-ones columns
    sel[:, 0] = 0.0
    sel[:, 1] = 0.0
    sel[0:K, 0] = 1.0
    sel[K:128, 1] = 1.0
    selt = np.zeros((128, 128), f16)                       # K=128 zero-padded
    selt[0, 0:K] = 1.0
    selt[1, K:128] = 1.0
    selt2 = np.zeros((128, 128), f16)                      # h1 variant (rows 32/33)
    selt2[32, 0:K] = 1.0
    selt2[33, K:128] = 1.0

    x8full = q8(x)                                         # (N, 256) fp8
    x2 = np.sum(x.astype(np.float32) ** 2, axis=1)         # (N,) f32
    r0 = q8(x2 * 0.5)                                      # 2-level fp8 x2:
    r1 = q8(x2 - 2.0 * r0.astype(np.float32))              # 2*r0 + r1 ~ x2

    in_maps = []
    for i in range(NCORES):
        sl = slice(i * NSH, (i + 1) * NSH)
        # [d, s, i, n] layout: DR k-tiles (d-halves) at stride SLAB
        xts = np.ascontiguousarray(
            x8full[sl].T.reshape(2, 128, NSLAB, SLAB)
            .transpose(1, 2, 0, 3).reshape(128, 2 * NSH))
        am = np.empty((1, NSLAB, 2, SLAB), f8)
        am[0, :, 0, :] = r0[sl].reshape(NSLAB, SLAB)
        am[0, :, 1, :] = r1[sl].reshape(NSLAB, SLAB)
        in_maps.append({"xt": xts, "augm": am.reshape(1, 2 * NSH),
                        "ctb": ctb, "augw": augw, "biasv": biasv,
                        "sel": sel, "selt": selt, "selt2": selt2})
    return in_maps


def _postprocess(results):
    """[128, NSH/2] f16 per core -> full [N, K] f32."""
    outs = []
    for r in results:
        qt = np.asarray(r["q"]).astype(np.float32)          # [128, 32768]
        a = qt.reshape(2, K, NSLAB, BPS, BLK // 2)          # [g, k, s, bi, j]
        outs.append(a.transpose(2, 3, 0, 4, 1).reshape(NSH, K))
    return np.concatenate(outs, axis=0)


def kernel(inputs, clusters):
    from concourse.bass_utils import run_bass_kernel_spmd

    nc = _get_nc()
    in_maps = _prep_inputs(inputs, clusters)
    res = run_bass_kernel_spmd(nc, in_maps, core_ids=list(range(NCORES)))
    return _postprocess(res.results)
